# revision 4
# baseline (speedup 1.0000x reference)
"""Trainium2 Bass kernel for an AttentionBlock (GroupNorm + 4-head self-attention + proj).

Sharding: 8 cores = 4 batches x 2 head-pairs. Core c handles batch c//2, heads
{2j, 2j+1} where j = c%2. Each core: groupnorm of x[b] (duplicated across the
pair of cores), QKV for its 128 feature channels, transposed-score flash
attention (no max subtraction -- scores are ~N(0,1)), per-head UNNORMALIZED
partial projection. The softmax denominators ship to the host, which applies
the per-(head, token) normalization to the per-head partial projections,
sums partials, and adds residual + proj bias.

Layout on device: features/keys on partitions, tokens on free dim.
  Q, K: fp8-e4m3 in a [32, h*2N + t*N + token] shuffle so QK runs as
        DoubleRow dual-pumped fp8 matmuls (107ns per (kt, head)).
  VT:   fp8-e5m2 token-major tiles (128 tokens, [V_A(64)|1|V_B(64)|1]) --
        attention output AND softmax denominator in one pass.
  probs: fp8-e5m2, written per 128-key tile by one of THREE engines:
        - ACT: activation(Exp, scale=1/8) with e5m2 output
        - DVE: Schraudolph bit-trick: u8 = rne(s*log2(e)/2 + 60) IS the e5m2
          bit pattern of exp(s/8) (linear-interp exp2; sat-at-0 kills the
          negative tail)
        - POOL: same Schraudolph tensor_scalar on the Pool/GpSimd engine.
        Three concurrent exp streams ~3x the softmax drain rate.
  PV:   DoubleRow fp8 matmuls: one matmul consumes TWO key tiles (256-deep
        contraction) at 0.5 cycles/row.
  No on-device softmax normalize: O rows 64 (the ones-column accumulators)
  are copied out as denominators; the projection is split per head (64-deep
  contractions) so the host can scale each head's partial by 1/denom.
"""
import sys

sys.path.insert(0, "/opt/trn_rl_repo")

import numpy as np

import concourse.bacc as bacc
import concourse.mybir as mybir
import concourse.tile as tile
from concourse import bass_utils

F32 = mybir.dt.float32
F32R = mybir.dt.float32r
BF16 = mybir.dt.bfloat16
E5 = mybir.dt.float8e5
E4 = mybir.dt.float8e4
U8 = mybir.dt.uint8
AF = mybir.ActivationFunctionType
ALU = mybir.AluOpType
AX = mybir.AxisListType
DR = mybir.MatmulPerfMode.DoubleRow

B, C, H, W = 4, 256, 64, 64
N = H * W                  # 4096 tokens
NG = 8                     # groupnorm groups
GSZ = C // NG              # 32 channels per group
NQC = 8                    # query chunks of 512
QC = 512
NKT = 32                   # key tiles of 128
KT = 128
NPR = NKT // 2             # kt pairs
INV_GN = 1.0 / GSZ         # stats are per-partition means
SCALE = 1.0 / 8.0          # hd^-0.5
A_SCH = np.log2(np.e) / 8.0 * 4.0   # schraudolph mult (folds 1/8 score scale)
B_SCH = 60.0                        # e5m2 exponent bias 15 << 2

# schedule knobs:
#   exp0 / exp: (act, dve, pool) exp-tile counts per qc (qc0 / rest)
#   fa_pos: sub-index where the finish_a op-list starts (one op per sub)
#   fb_pos: sub-index where the finish_b op-list starts (one op per sub)
#   q_pos:  sub-index where next q chunk-pair is emitted (even qc only)
#   o_eng / dn_eng / y_eng: engine for O->SBUF, denom-row, y PSUM->SBUF moves
KNOBS = dict(exp0=(13, 12, 7), exp=(11, 11, 10), fa_pos=3, fb_pos=15,
             q_pos=16, pop_lo=2, pop_len=3, qk_mover="act",
             o_eng="pool", dn_eng="dve", y_eng="act", vt_split=32,
             wq="sync", kshq="sync", qshq="sync")

_CACHE: dict = {}


def _exp_pattern(counts):
    """Interleave engine assignments for 32 kt tiles by largest remainder."""
    names = ["act", "dve", "pool"]
    cnt = dict(zip(names, counts))
    assigned = {e: 0 for e in names}
    pat = []
    for k in range(NKT):
        best, be = -1e9, None
        for e in names:
            if cnt[e] <= 0:
                continue
            err = cnt[e] * (k + 1) / NKT - assigned[e]
            if err > best:
                best, be = err, e
        pat.append(be)
        assigned[be] += 1
    return pat


def _build():
    nc = bacc.Bacc("TRN2", target_bir_lowering=False, debug=False,
                   enable_asserts=False)

    xb = nc.dram_tensor("xb", [8, 128, 1024], BF16, kind="ExternalInput")
    wslb = nc.dram_tensor("wslb", [2, 128, 400], BF16, kind="ExternalInput")
    csts = nc.dram_tensor("csts", [128, 10], F32, kind="ExternalInput")
    bv16 = nc.dram_tensor("bv16", [1, 144], BF16, kind="ExternalInput")
    selt = nc.dram_tensor("selt", [4, 128], F32, kind="ExternalInput")
    wpt = nc.dram_tensor("wpt", [128, 256], BF16, kind="ExternalInput")
    yp = nc.dram_tensor("yp", [16, 128, 1024], F32, kind="ExternalOutput")
    dnp = nc.dram_tensor("dnp", [2, 4096], F32, kind="ExternalOutput")

    pat0 = _exp_pattern(KNOBS["exp0"])
    pat = _exp_pattern(KNOBS["exp"])

    with tile.TileContext(nc) as tc:
        with (
            tc.tile_pool(name="cst", bufs=1) as cst,
            tc.tile_pool(name="big", bufs=1) as big,
            tc.tile_pool(name="pp", bufs=10) as pp,
            tc.tile_pool(name="sm", bufs=3) as sm,
            tc.tile_pool(name="yy", bufs=3) as yy,
            tc.tile_pool(name="ps", bufs=3, space="PSUM") as ps,
            tc.tile_pool(name="po", bufs=2, space="PSUM") as po,
        ):
            # ---- constants ----
            W0 = cst.tile([128, 400], BF16, tag="w0")
            W1 = cst.tile([128, 400], BF16, tag="w1")
            WPA = cst.tile([64, 256], BF16, tag="wpa")
            WPB = cst.tile([64, 256], BF16, tag="wpb")
            CST = cst.tile([128, 10], F32, tag="cst")
            BQK = CST[:, 0:2]
            GAM = CST[:, 2:4]
            BET = CST[:, 4:6]
            SEL = CST[:, 6:10]
            BV = cst.tile([1, 144], BF16, tag="bv")
            SELT = cst.tile([4, 128], F32, tag="selt")
            ONE = cst.tile([1, 128], F32, tag="one")
            ONEB = cst.tile([1, 128], BF16, tag="oneb")
            EPS = cst.tile([128, 1], F32, tag="eps")
            DN = cst.tile([2, 4096], F32, tag="dn")

            # prime the sqrt act table before the stats activations (all of
            # Identity/Square live in every table) so the groupnorm Sqrt and
            # the stats run without any mid-prologue table reloads.
            WARM = cst.tile([1, 1], F32, tag="warm")
            nc.vector.memset(WARM[:], 1.0)
            nc.scalar.activation(WARM[:], WARM[:], AF.Sqrt)

            # ---- load x (chunked, stats via one-pass bn_stats) ----
            NCH = 8
            CH = N // NCH   # 512
            X = [big.tile([128, N], BF16, tag=f"x{cc}", name=f"X{cc}") for cc in range(2)]
            Hb = [big.tile([128, N], BF16, tag=f"hb{cc}", name=f"Hb{cc}") for cc in range(2)]
            BNS = [cst.tile([128, NCH * 6], F32, tag=f"bns{cc}", name=f"BNS{cc}") for cc in range(2)]
            MV = [cst.tile([128, 2], F32, tag=f"mv{cc}", name=f"MV{cc}") for cc in range(2)]
            ST = [cst.tile([128, 2], F32, tag=f"st{cc}", name=f"ST{cc}") for cc in range(2)]
            GS = cst.tile([4, 4], F32, tag="gs")
            gs_ps = po.tile([4, 4], F32, tag="po")
            SX = cst.tile([128, 4], F32, tag="sx")
            SQ = cst.tile([128, 4], F32, tag="sq")
            for i in range(4):
                for cc in range(2):
                    dsl = slice(i * 1024, (i + 1) * 1024)
                    xq = nc.sync if cc == 0 else nc.gpsimd
                    xq.dma_start(X[cc][:, dsl], xb.ap()[cc * 4 + i])
                    for h in range(2):
                        j = 2 * i + h
                        sl = slice(j * CH, (j + 1) * CH)
                        if cc == 1 and j < 4:
                            # half-1 stats on ACT (idle early); scratch into
                            # Hb[1] (overwritten later by real Hb)
                            nc.scalar.activation(
                                Hb[1][:, sl], X[1][:, sl], AF.Identity,
                                accum_out=SX[:, j:j + 1])
                            nc.scalar.activation(
                                Hb[1][:, sl], X[1][:, sl], AF.Square,
                                accum_out=SQ[:, j:j + 1])
                        else:
                            nc.vector.bn_stats(BNS[cc][:, 6 * j:6 * j + 6],
                                               X[cc][:, sl])
            # weights & consts (needed later than x)
            nc.vector.memset(EPS[:], 1e-5)
            nc.vector.memset(ONE[:], 1.0)
            nc.vector.memset(ONEB[:], 1.0)
            WQ = {"sync": nc.sync, "gpsimd": nc.gpsimd, "scalar": nc.scalar}[KNOBS["wq"]]
            WQ.dma_start(CST[:], csts.ap())
            WQ.dma_start(BV[:], bv16.ap())
            WQ.dma_start(SELT[:], selt.ap())
            WQ.dma_start(W0[:], wslb.ap()[0])
            WQ.dma_start(W1[:], wslb.ap()[1])
            WQ.dma_start(WPA[:], wpt.ap()[0:64])
            WQ.dma_start(WPB[:], wpt.ap()[64:128])
            for cc in range(2):
                if cc == 0:
                    nc.vector.bn_aggr(MV[0][:], BNS[0][:])
                    # ST = [mean_p, E[x^2]_p]
                    nc.vector.tensor_mul(ST[0][:, 1:2], MV[0][:, 0:1], MV[0][:, 0:1])
                    nc.vector.tensor_add(ST[0][:, 1:2], ST[0][:, 1:2], MV[0][:, 1:2])
                    nc.vector.tensor_copy(ST[0][:, 0:1], MV[0][:, 0:1])
                else:
                    # merge ACT sums (slices 0-3) with bn stats (slices 4-7)
                    nc.vector.bn_aggr(MV[1][:], BNS[1][:, 24:48])
                    sxs = cst.tile([128, 1], F32, tag="sxs")
                    sqs = cst.tile([128, 1], F32, tag="sqs")
                    nc.vector.reduce_sum(sxs[:], SX[:], axis=AX.X)
                    nc.vector.reduce_sum(sqs[:], SQ[:], axis=AX.X)
                    # mean_p = 0.5*mean_bn + sxs/4096
                    nc.vector.tensor_scalar_mul(ST[1][:, 0:1], MV[1][:, 0:1], 0.5)
                    nc.vector.tensor_scalar_mul(sxs[:], sxs[:], 1.0 / 4096.0)
                    nc.vector.tensor_add(ST[1][:, 0:1], ST[1][:, 0:1], sxs[:])
                    # E2_p = 0.5*(var_bn + mean_bn^2) + sqs/4096
                    nc.vector.tensor_mul(ST[1][:, 1:2], MV[1][:, 0:1], MV[1][:, 0:1])
                    nc.vector.tensor_add(ST[1][:, 1:2], ST[1][:, 1:2], MV[1][:, 1:2])
                    nc.vector.tensor_scalar_mul(ST[1][:, 1:2], ST[1][:, 1:2], 0.5)
                    nc.vector.tensor_scalar_mul(sqs[:], sqs[:], 1.0 / 4096.0)
                    nc.vector.tensor_add(ST[1][:, 1:2], ST[1][:, 1:2], sqs[:])
                nc.tensor.matmul(gs_ps[:, 2 * cc:2 * cc + 2], SEL,
                                 ST[cc][:], start=True, stop=True)
                nc.vector.tensor_copy(GS[:, 2 * cc:2 * cc + 2],
                                      gs_ps[:, 2 * cc:2 * cc + 2])

            # per-channel scale/shift: s = gamma/sqrt(var+eps), t = beta - mean*s
            gn_st = []
            for cc in range(2):
                pc_ps = po.tile([128, 2], F32, tag="po")
                nc.tensor.matmul(pc_ps[:], SELT[:], GS[:, 2 * cc:2 * cc + 2],
                                 start=True, stop=True)
                mean = cst.tile([128, 1], F32, tag=f"mean{cc}")
                var = cst.tile([128, 1], F32, tag=f"var{cc}")
                sd = cst.tile([128, 1], F32, tag=f"sd{cc}")
                s_t = cst.tile([128, 1], F32, tag=f"s{cc}")
                t_t = cst.tile([128, 1], F32, tag=f"t{cc}")
                nc.vector.tensor_scalar_mul(mean[:], pc_ps[:, 0:1], INV_GN)
                nc.vector.tensor_scalar_mul(var[:], pc_ps[:, 1:2], INV_GN)
                # var = E[x^2] - mean^2
                nc.vector.scalar_tensor_tensor(
                    out=sd[:], in0=mean[:], scalar=-1.0, in1=mean[:],
                    op0=ALU.mult, op1=ALU.mult)
                nc.vector.tensor_add(var[:], var[:], sd[:])
                nc.scalar.activation(sd[:], var[:], AF.Sqrt, bias=EPS[:])
                nc.vector.reciprocal(s_t[:], sd[:])
                nc.vector.tensor_mul(s_t[:], s_t[:], GAM[:, cc:cc + 1])
                nc.vector.scalar_tensor_tensor(
                    out=t_t[:], in0=mean[:], scalar=-1.0, in1=s_t[:],
                    op0=ALU.mult, op1=ALU.mult)
                nc.vector.tensor_add(t_t[:], t_t[:], BET[:, cc:cc + 1])
                gn_st.append((s_t, t_t))

            # h = x*s + t (bf16): all on DVE -- all-SBUF 2-byte TensorScalar
            # hits the 4x DVE mode (~327ns/slice)
            nc.scalar.activation(WARM[:], WARM[:], AF.Exp)  # preload exp table
            def emit_hb(i):
                sl = slice(i * 1024, (i + 1) * 1024)
                for cc in range(2):
                    s_t, t_t = gn_st[cc]
                    nc.vector.tensor_scalar(
                        out=Hb[cc][:, sl], in0=X[cc][:, sl], scalar1=s_t[:],
                        scalar2=t_t[:], op0=ALU.mult, op1=ALU.add)
            emit_hb(0)

            # ---- QKV ----
            # Q2/K2: fp8-e4m3, head-dim split across 2 DoubleRow k-subtiles:
            # partition r (0..31), free = h*8192 + t*4096 + token, where the
            # original feature index is h*64 + t*32 + r. QK then runs as one
            # dual-pumped fp8 matmul per (kt, head): 107ns instead of 427.
            Q2 = big.tile([32, 4 * N], E4, tag="q2")
            K2 = big.tile([32, 4 * N], E4, tag="k2")
            VT = big.tile([128, NKT * 144], E5, tag="vt")

            # staging holds a PAIR of 512-token chunks -> 4 shuffle DMAs per
            # pair instead of per chunk
            Q8 = [big.tile([128, 2 * QC], E4, tag=f"q8_{i}", name=f"Q8_{i}")
                  for i in range(2)]
            K8 = [big.tile([128, 2 * QC], E4, tag=f"k8_{i}", name=f"K8_{i}")
                  for i in range(2)]

            def _shuffle2(dst2, stage, chpair, q):
                # (128, 1024) staging of chunks {2c, 2c+1} -> (32, [h][t])
                for h in range(2):
                    for t in range(2):
                        off = (h * 2 + t) * N + chpair * 2 * QC
                        q.dma_start(dst2[:, off:off + 2 * QC],
                                    stage[h * 64 + t * 32:h * 64 + t * 32 + 32, :])

            qs_map = {"sync": nc.sync, "gpsimd": nc.gpsimd}

            def _mover(stage_sl, src_ps, bias_ap):
                if KNOBS["qk_mover"] == "dve":
                    nc.vector.tensor_scalar(out=stage_sl, in0=src_ps,
                                            scalar1=bias_ap, scalar2=None,
                                            op0=ALU.add)
                elif KNOBS["qk_mover"] == "pool":
                    nc.gpsimd.tensor_scalar(out=stage_sl, in0=src_ps,
                                            scalar1=bias_ap, scalar2=None,
                                            op0=ALU.add)
                else:
                    nc.scalar.activation(stage_sl, src_ps, AF.Identity,
                                         bias=bias_ap)

            def emit_q_pair(chpair):
                stage = Q8[chpair % 2]
                for sub in range(2):
                    ch = 2 * chpair + sub
                    tok = slice(ch * QC, (ch + 1) * QC)
                    q_ps = ps.tile([128, QC], F32, tag="s", name=f"q_ps{ch}")
                    nc.tensor.matmul(q_ps[:], W0[:, 0:128], Hb[0][:, tok],
                                     start=True, stop=False)
                    nc.tensor.matmul(q_ps[:], W1[:, 0:128], Hb[1][:, tok],
                                     start=False, stop=True)
                    _mover(stage[:, sub * QC:(sub + 1) * QC], q_ps[:],
                           BQK[:, 0:1])
                _shuffle2(Q2, stage, chpair, qs_map[KNOBS["qshq"]])

            def emit_k_pair(chpair):
                stage = K8[chpair % 2]
                for sub in range(2):
                    ch = 2 * chpair + sub
                    tok = slice(ch * QC, (ch + 1) * QC)
                    k_ps = ps.tile([128, QC], F32, tag="s", name=f"k_ps{ch}")
                    nc.tensor.matmul(k_ps[:], W0[:, 128:256], Hb[0][:, tok],
                                     start=True, stop=False)
                    nc.tensor.matmul(k_ps[:], W1[:, 128:256], Hb[1][:, tok],
                                     start=False, stop=True)
                    _mover(stage[:, sub * QC:(sub + 1) * QC], k_ps[:],
                           BQK[:, 1:2])
                _shuffle2(K2, stage, chpair, qs_map[KNOBS["kshq"]])

            def emit_vt_tile(kt):
                tok = slice(kt * KT, (kt + 1) * KT)
                vt_ps = ps.tile([128, 144], F32, tag="s", name=f"vt_ps{kt}")
                nc.tensor.matmul(vt_ps[:], Hb[0][:, tok], W0[:, 256:400],
                                 start=True, stop=False)
                nc.tensor.matmul(vt_ps[:], Hb[1][:, tok], W1[:, 256:400],
                                 start=False, stop=False)
                # V bias (+ the denominator 1s column) via rank-1 accumulate
                nc.tensor.matmul(vt_ps[:], ONEB[0:1, :], BV[:],
                                 start=False, stop=True)
                nc.gpsimd.tensor_copy(VT[:, kt * 144:(kt + 1) * 144], vt_ps[:])

            # chunk-pair-0 QKV only needs Hb cols 0:1024 -- start right after
            # the first Hb slice, then fill in the rest of Hb
            emit_q_pair(0)
            emit_k_pair(0)
            for i in range(1, 4):
                emit_hb(i)

            # ---- attention + per-head unnormalized projection ----
            ENG = {"act": None, "dve": nc.vector, "pool": nc.gpsimd}

            def _copy(eng, dst, src):
                if eng == "act":
                    nc.scalar.activation(dst, src, AF.Copy)
                elif eng == "dve":
                    nc.vector.tensor_copy(dst, src)
                else:
                    nc.gpsimd.tensor_copy(dst, src)

            pending = []     # finish ops of the previous qc, one per sub slot
            pv_queue = []
            for qc in range(NQC):
                O_A = po.tile([72, QC], F32, tag="po", name=f"O_A{qc}")
                O_B = po.tile([72, QC], F32, tag="po", name=f"O_B{qc}")
                mypat = pat0 if qc == 0 else pat
                for pr in range(NPR):
                    P8 = pp.tile([128, 2048], E5, tag="p", name=f"p{qc}_{pr}")
                    for sub in range(2):
                        kt = 2 * pr + sub
                        if qc == 0:
                            if kt % 8 == 4 and kt // 8 < 3:
                                emit_k_pair(kt // 8 + 1)
                            if kt < KNOBS["vt_split"]:
                                emit_vt_tile(kt)
                        if qc == 1 and kt < NKT - KNOBS["vt_split"]:
                            emit_vt_tile(kt + KNOBS["vt_split"])
                        if pr <= 1 and pv_queue:
                            pv_queue.pop(0)()
                        si = 2 * pr + sub
                        for pos, op in list(pending):
                            if si >= pos:
                                op()
                                pending.remove((pos, op))
                        if si == KNOBS["q_pos"] and qc % 2 == 0 and qc // 2 + 1 < 4:
                            emit_q_pair(qc // 2 + 1)
                        if pr >= KNOBS["pop_lo"] and len(pv_queue) >= KNOBS["pop_len"]:
                            pv_queue.pop(0)()
                        s_ps = ps.tile([128, 1024], F32, tag="s",
                                       name=f"s{qc}_{kt}")
                        if qc == 0 and kt < 8:
                            # warmup: the un-shuffled e4m3 staging tiles still
                            # hold these key chunks -- plain fp8 matmuls, no
                            # wait on the shuffle DMAs
                            ks8 = K8[0]
                            kk = slice(kt * KT, (kt + 1) * KT)
                            nc.tensor.matmul(s_ps[:, 0:512], ks8[0:64, kk],
                                             Q8[0][0:64, 0:QC], start=True,
                                             stop=True)
                            nc.tensor.matmul(s_ps[:, 512:1024], ks8[64:128, kk],
                                             Q8[0][64:128, 0:QC], start=True,
                                             stop=True)
                        else:
                            k2v = K2[:].rearrange("p (h t c) -> h p t c", h=2, t=2)[
                                :, :, :, kt * KT:(kt + 1) * KT]
                            q2v = Q2[:].rearrange("p (h t c) -> h p t c", h=2, t=2)[
                                :, :, :, qc * QC:(qc + 1) * QC]
                            nc.tensor.matmul(s_ps[:, 0:512], k2v[0], q2v[0],
                                             start=True, stop=True, perf_mode=DR)
                            nc.tensor.matmul(s_ps[:, 512:1024], k2v[1], q2v[1],
                                             start=True, stop=True, perf_mode=DR)
                        dst = P8[:, sub * 1024:(sub + 1) * 1024]
                        eng = mypat[kt]
                        if eng == "dve":
                            nc.vector.tensor_scalar(
                                out=dst.bitcast(U8), in0=s_ps[:],
                                scalar1=A_SCH, scalar2=B_SCH,
                                op0=ALU.mult, op1=ALU.add)
                        elif eng == "pool":
                            nc.gpsimd.tensor_scalar(
                                out=dst.bitcast(U8), in0=s_ps[:],
                                scalar1=A_SCH, scalar2=B_SCH,
                                op0=ALU.mult, op1=ALU.add)
                        else:
                            nc.scalar.activation(dst, s_ps[:], AF.Exp,
                                                 scale=SCALE)

                    def _pv(pr=pr, P8=P8, O_A=O_A, O_B=O_B):
                        vt_ap = VT[:].rearrange("p (t x) -> p t x", t=NKT)[
                            :, 2 * pr:2 * pr + 2, :]
                        p_ap = P8[:].rearrange("p (t x) -> p t x", t=2)
                        nc.tensor.matmul(O_A[:], vt_ap[:, :, 0:72],
                                         p_ap[:, :, 0:512],
                                         start=(pr == 0), stop=(pr == NPR - 1),
                                         perf_mode=DR)
                        nc.tensor.matmul(O_B[:], vt_ap[:, :, 72:144],
                                         p_ap[:, :, 512:1024],
                                         start=(pr == 0), stop=(pr == NPR - 1),
                                         perf_mode=DR)
                    pv_queue.append(_pv)

                OsbA = sm.tile([64, QC], BF16, tag="osba", name=f"OsbA{qc}")
                OsbB = sm.tile([64, QC], BF16, tag="osbb", name=f"OsbB{qc}")

                def fa0(qc=qc, O_A=O_A, OsbA=OsbA):
                    _copy(KNOBS["o_eng"], OsbA[:], O_A[0:64, :])

                def fa1(qc=qc, O_B=O_B, OsbB=OsbB):
                    _copy(KNOBS["o_eng"], OsbB[:], O_B[0:64, :])

                def fdn(qc=qc, O_A=O_A, O_B=O_B):
                    dsl = slice(qc * QC, (qc + 1) * QC)
                    _copy(KNOBS["dn_eng"], DN[0:1, dsl], O_A[64:65, :])
                    _copy(KNOBS["dn_eng"], DN[1:2, dsl], O_B[64:65, :])

                def fy(half, qc=qc, OsbA=OsbA, OsbB=OsbB):
                    cols = slice(half * 128, (half + 1) * 128)
                    y_ps = ps.tile([128, 1024], F32, tag="s",
                                   name=f"y_ps{qc}_{half}")
                    nc.tensor.matmul(y_ps[:, 0:512], WPA[:, cols],
                                     OsbA[:], start=True, stop=True)
                    nc.tensor.matmul(y_ps[:, 512:1024], WPB[:, cols],
                                     OsbB[:], start=True, stop=True)
                    y_sb = yy.tile([128, 1024], F32, tag="y",
                                   name=f"y_sb{qc}_{half}")
                    ye = KNOBS["y_eng"]
                    if ye == "split":
                        ye = "act" if half == 0 else "dve"
                    _copy(ye, y_sb[:], y_ps[:])
                    nc.sync.dma_start(yp.ap()[half * 8 + qc], y_sb[:])

                fa, fb = KNOBS["fa_pos"], KNOBS["fb_pos"]
                mine = [(fa, fa0), (fa + 1, fa1), (fa + 2, fdn),
                        (fb, lambda qc=qc, OsbA=OsbA, OsbB=OsbB: fy(0, qc, OsbA, OsbB)),
                        (fb + 1, lambda qc=qc, OsbA=OsbA, OsbB=OsbB: fy(1, qc, OsbA, OsbB))]
                if qc < NQC - 1:
                    pending = mine
                else:
                    while pv_queue:
                        pv_queue.pop(0)()
                    for _, op in mine:
                        op()
            nc.sync.dma_start(dnp.ap(), DN[:])

    nc.compile()
    return nc


def _get_nc():
    if "nc" not in _CACHE:
        _CACHE["nc"] = _build()
    return _CACHE["nc"]


def build_in_maps(x, gn_gamma, gn_beta, w_qkv, b_qkv, w_proj):
    import ml_dtypes
    sel_np = np.zeros((128, 4), np.float32)
    for c in range(128):
        sel_np[c, c // 32] = 1.0
    selt_np = sel_np.T.copy()
    gmt_np = np.stack([gn_gamma[0:128], gn_gamma[128:256]], axis=1)
    btt_np = np.stack([gn_beta[0:128], gn_beta[128:256]], axis=1)

    in_maps = []
    for core in range(8):
        b, j = core // 2, core % 2
        r0 = 128 * j
        wsl_np = np.zeros((2, 128, 400), np.float32)
        for cc in range(2):
            cols = slice(cc * 128, (cc + 1) * 128)
            wsl_np[cc, :, 0:128] = w_qkv[r0:r0 + 128, cols].T
            wsl_np[cc, :, 128:256] = w_qkv[256 + r0:256 + r0 + 128, cols].T
            wsl_np[cc, :, 256:320] = w_qkv[512 + r0:512 + r0 + 64, cols].T
            wsl_np[cc, :, 328:392] = w_qkv[512 + r0 + 64:512 + r0 + 128, cols].T
        bqk_np = np.stack([b_qkv[r0:r0 + 128], b_qkv[256 + r0:256 + r0 + 128]],
                          axis=1)
        bv_np = np.zeros((1, 144), np.float32)
        bv_np[0, 0:64] = b_qkv[512 + r0:512 + r0 + 64]
        bv_np[0, 64] = 1.0
        bv_np[0, 72:136] = b_qkv[512 + r0 + 64:512 + r0 + 128]
        bv_np[0, 136] = 1.0
        csts_np = np.concatenate([bqk_np, gmt_np, btt_np, sel_np], axis=1)
        xq = np.ascontiguousarray(
            x[b].reshape(2, 128, 4, 1024).transpose(0, 2, 1, 3)
            .reshape(8, 128, 1024).astype(ml_dtypes.bfloat16))
        in_maps.append({
            "xb": xq,
            "wslb": np.ascontiguousarray(wsl_np.astype(ml_dtypes.bfloat16)),
            "csts": np.ascontiguousarray(csts_np),
            "bv16": np.ascontiguousarray(bv_np.astype(ml_dtypes.bfloat16)),
            "selt": selt_np,
            "wpt": np.ascontiguousarray(
                w_proj[:, r0:r0 + 128].T.astype(ml_dtypes.bfloat16)),
        })

    return in_maps


def kernel(x, gn_gamma, gn_beta, w_qkv, b_qkv, w_proj, b_proj, **_unused):
    x = np.ascontiguousarray(np.asarray(x, dtype=np.float32))
    gn_gamma = np.asarray(gn_gamma, dtype=np.float32)
    gn_beta = np.asarray(gn_beta, dtype=np.float32)
    w_qkv = np.asarray(w_qkv, dtype=np.float32)
    b_qkv = np.asarray(b_qkv, dtype=np.float32)
    w_proj = np.asarray(w_proj, dtype=np.float32)
    b_proj = np.asarray(b_proj, dtype=np.float32)

    nc = _get_nc()
    in_maps = build_in_maps(x, gn_gamma, gn_beta, w_qkv, b_qkv, w_proj)
    res = bass_utils.run_bass_kernel_spmd(nc, in_maps, core_ids=list(range(8)))
    _CACHE["last_result"] = res

    out = np.empty((B, C, N), np.float32)
    for b in range(B):
        acc = np.zeros((C, N), np.float32)
        for j in range(2):
            r = res.results[2 * b + j]
            ypj = r["yp"].reshape(2, 8, 128, 2, 512)   # half, qc, ch, head, tok
            rden = 1.0 / r["dnp"]                       # (2, 4096) head, tok
            # per head: (half, qc, ch, tok) -> (C, N), scaled by 1/denom
            for head in range(2):
                yh = ypj[:, :, :, head, :]              # (2, 8, 128, 512)
                yh = yh.transpose(0, 2, 1, 3).reshape(C, N)
                acc += yh * rden[head][None, :]
        out[b] = acc + x[b].reshape(C, N) + b_proj[:, None]
    return out.reshape(B, C, H, W)


# revision 6
# speedup vs baseline: 1.1302x; 1.1302x over previous
"""Trainium2 Bass kernel for an AttentionBlock (GroupNorm + 4-head self-attention + proj).

Sharding: 8 cores = 4 batches x 2 head-pairs. Core c handles batch c//2, heads
{2j, 2j+1} where j = c%2. Each core: groupnorm of x[b] (duplicated across the
pair of cores), QKV for its 128 feature channels, transposed-score flash
attention (no max subtraction -- scores are ~N(0,1)), per-head UNNORMALIZED
partial projection. The softmax denominators ship to the host, which applies
the per-(head, token) normalization to the per-head partial projections,
sums partials, and adds residual + proj bias.

Layout on device: features/keys on partitions, tokens on free dim.
  Q, K: fp8-e4m3 in a [32, h*2N + t*N + token] shuffle so QK runs as
        DoubleRow dual-pumped fp8 matmuls (107ns per (kt, head)).
  VT:   fp8-e5m2 token-major tiles (128 tokens, [V_A(64)|1|V_B(64)|1]) --
        attention output AND softmax denominator in one pass.
  probs: fp8-e5m2, written per 128-key tile by one of THREE engines:
        - ACT: activation(Exp, scale=1/8) with e5m2 output
        - DVE: Schraudolph bit-trick: u8 = rne(s*log2(e)/2 + 60) IS the e5m2
          bit pattern of exp(s/8) (linear-interp exp2; sat-at-0 kills the
          negative tail)
        - POOL: same Schraudolph tensor_scalar on the Pool/GpSimd engine.
        Three concurrent exp streams ~3x the softmax drain rate.
  PV:   DoubleRow fp8 matmuls: one matmul consumes TWO key tiles (256-deep
        contraction) at 0.5 cycles/row.
  No on-device softmax normalize: O rows 64 (the ones-column accumulators)
  are copied out as denominators; the projection is split per head (64-deep
  contractions) so the host can scale each head's partial by 1/denom.
"""
import sys

sys.path.insert(0, "/opt/trn_rl_repo")

import numpy as np

import concourse.bacc as bacc
import concourse.mybir as mybir
import concourse.tile as tile
from concourse import bass_utils

F32 = mybir.dt.float32
F32R = mybir.dt.float32r
BF16 = mybir.dt.bfloat16
E5 = mybir.dt.float8e5
E4 = mybir.dt.float8e4
U8 = mybir.dt.uint8
AF = mybir.ActivationFunctionType
ALU = mybir.AluOpType
AX = mybir.AxisListType
DR = mybir.MatmulPerfMode.DoubleRow

B, C, H, W = 4, 256, 64, 64
N = H * W                  # 4096 tokens
NG = 8                     # groupnorm groups
GSZ = C // NG              # 32 channels per group
NQC = 8                    # query chunks of 512
QC = 512
NKT = 32                   # key tiles of 128
KT = 128
NPR = NKT // 2             # kt pairs
INV_GN = 1.0 / GSZ         # stats are per-partition means
SCALE = 1.0 / 8.0          # hd^-0.5
A_SCH = np.log2(np.e) / 8.0 * 4.0   # schraudolph mult (folds 1/8 score scale)
B_SCH = 60.0                        # e5m2 exponent bias 15 << 2

# schedule knobs:
#   exp0 / exp: (act, dve, pool) exp-tile counts per qc (qc0 / rest)
#   fa_pos: sub-index where the finish_a op-list starts (one op per sub)
#   fb_pos: sub-index where the finish_b op-list starts (one op per sub)
#   q_pos:  sub-index where next q chunk-pair is emitted (even qc only)
#   o_eng / dn_eng / y_eng: engine for O->SBUF, denom-row, y PSUM->SBUF moves
KNOBS = dict(exp0=(28, 24, 12), exp=(27, 24, 13), fa_pos=3, fb_pos=15,
             q_pos=16, pop_lo=2, pop_len=3, qk_mover="act",
             o_eng="pool", dn_eng="pool", y_eng="split", vt_split=32,
             wq="sync", kshq="sync", qshq="sync", nslots=64)

_CACHE: dict = {}


def _exp_pattern(counts, nslots=64):
    """Interleave engine assignments for nslots half-tiles by largest
    remainder."""
    names = ["act", "dve", "pool"]
    cnt = dict(zip(names, counts))
    assigned = {e: 0 for e in names}
    pat = []
    for k in range(nslots):
        best, be = -1e9, None
        for e in names:
            if cnt[e] <= 0:
                continue
            err = cnt[e] * (k + 1) / nslots - assigned[e]
            if err > best:
                best, be = err, e
        pat.append(be)
        assigned[be] += 1
    return pat


def _build():
    nc = bacc.Bacc("TRN2", target_bir_lowering=False, debug=False,
                   enable_asserts=False)

    xb = nc.dram_tensor("xb", [8, 128, 1024], BF16, kind="ExternalInput")
    wslb = nc.dram_tensor("wslb", [2, 128, 400], BF16, kind="ExternalInput")
    csts = nc.dram_tensor("csts", [128, 10], F32, kind="ExternalInput")
    bv16 = nc.dram_tensor("bv16", [1, 144], BF16, kind="ExternalInput")
    selt = nc.dram_tensor("selt", [4, 128], F32, kind="ExternalInput")
    wpt = nc.dram_tensor("wpt", [128, 256], BF16, kind="ExternalInput")
    yp = nc.dram_tensor("yp", [16, 128, 1024], F32, kind="ExternalOutput")
    dnp = nc.dram_tensor("dnp", [2, 4096], F32, kind="ExternalOutput")

    pat0 = _exp_pattern(KNOBS["exp0"])
    pat = _exp_pattern(KNOBS["exp"])

    with tile.TileContext(nc) as tc:
        with (
            tc.tile_pool(name="cst", bufs=1) as cst,
            tc.tile_pool(name="big", bufs=1) as big,
            tc.tile_pool(name="pp", bufs=10) as pp,
            tc.tile_pool(name="sm", bufs=3) as sm,
            tc.tile_pool(name="yy", bufs=3) as yy,
            tc.tile_pool(name="ps", bufs=6, space="PSUM") as ps,
            tc.tile_pool(name="po", bufs=2, space="PSUM") as po,
        ):
            # ---- constants ----
            W0 = cst.tile([128, 400], BF16, tag="w0")
            W1 = cst.tile([128, 400], BF16, tag="w1")
            WPA = cst.tile([64, 256], BF16, tag="wpa")
            WPB = cst.tile([64, 256], BF16, tag="wpb")
            CST = cst.tile([128, 10], F32, tag="cst")
            BQK = CST[:, 0:2]
            GAM = CST[:, 2:4]
            BET = CST[:, 4:6]
            SEL = CST[:, 6:10]
            BV = cst.tile([1, 144], BF16, tag="bv")
            SELT = cst.tile([4, 128], F32, tag="selt")
            ONE = cst.tile([1, 128], F32, tag="one")
            ONEB = cst.tile([1, 128], BF16, tag="oneb")
            EPS = cst.tile([128, 1], F32, tag="eps")
            DN = cst.tile([2, 4096], F32, tag="dn")

            # prime the sqrt act table before the stats activations (all of
            # Identity/Square live in every table) so the groupnorm Sqrt and
            # the stats run without any mid-prologue table reloads.
            WARM = cst.tile([1, 1], F32, tag="warm")
            nc.vector.memset(WARM[:], 1.0)
            nc.scalar.activation(WARM[:], WARM[:], AF.Sqrt)

            # ---- load x (chunked, stats via one-pass bn_stats) ----
            NCH = 8
            CH = N // NCH   # 512
            X = [big.tile([128, N], BF16, tag=f"x{cc}", name=f"X{cc}") for cc in range(2)]
            Hb = [big.tile([128, N], BF16, tag=f"hb{cc}", name=f"Hb{cc}") for cc in range(2)]
            BNS = [cst.tile([128, NCH * 6], F32, tag=f"bns{cc}", name=f"BNS{cc}") for cc in range(2)]
            MV = [cst.tile([128, 2], F32, tag=f"mv{cc}", name=f"MV{cc}") for cc in range(2)]
            ST = [cst.tile([128, 2], F32, tag=f"st{cc}", name=f"ST{cc}") for cc in range(2)]
            GS = cst.tile([4, 4], F32, tag="gs")
            gs_ps = po.tile([4, 4], F32, tag="po")
            SX = cst.tile([128, 4], F32, tag="sx")
            SQ = cst.tile([128, 4], F32, tag="sq")
            for i in range(4):
                for cc in range(2):
                    dsl = slice(i * 1024, (i + 1) * 1024)
                    xq = nc.sync if cc == 0 else nc.gpsimd
                    xq.dma_start(X[cc][:, dsl], xb.ap()[cc * 4 + i])
                    for h in range(2):
                        j = 2 * i + h
                        sl = slice(j * CH, (j + 1) * CH)
                        if cc == 1 and j < 4:
                            # half-1 stats on ACT (idle early); scratch into
                            # Hb[1] (overwritten later by real Hb)
                            nc.scalar.activation(
                                Hb[1][:, sl], X[1][:, sl], AF.Identity,
                                accum_out=SX[:, j:j + 1])
                            nc.scalar.activation(
                                Hb[1][:, sl], X[1][:, sl], AF.Square,
                                accum_out=SQ[:, j:j + 1])
                        else:
                            nc.vector.bn_stats(BNS[cc][:, 6 * j:6 * j + 6],
                                               X[cc][:, sl])
            # weights & consts (needed later than x)
            nc.vector.memset(EPS[:], 1e-5)
            nc.vector.memset(ONE[:], 1.0)
            nc.vector.memset(ONEB[:], 1.0)
            WQ = {"sync": nc.sync, "gpsimd": nc.gpsimd, "scalar": nc.scalar}[KNOBS["wq"]]
            WQ.dma_start(CST[:], csts.ap())
            WQ.dma_start(BV[:], bv16.ap())
            WQ.dma_start(SELT[:], selt.ap())
            WQ.dma_start(W0[:], wslb.ap()[0])
            WQ.dma_start(W1[:], wslb.ap()[1])
            WQ.dma_start(WPA[:], wpt.ap()[0:64])
            WQ.dma_start(WPB[:], wpt.ap()[64:128])
            for cc in range(2):
                if cc == 0:
                    nc.vector.bn_aggr(MV[0][:], BNS[0][:])
                    # ST = [mean_p, E[x^2]_p]
                    nc.vector.tensor_mul(ST[0][:, 1:2], MV[0][:, 0:1], MV[0][:, 0:1])
                    nc.vector.tensor_add(ST[0][:, 1:2], ST[0][:, 1:2], MV[0][:, 1:2])
                    nc.vector.tensor_copy(ST[0][:, 0:1], MV[0][:, 0:1])
                else:
                    # merge ACT sums (slices 0-3) with bn stats (slices 4-7)
                    nc.vector.bn_aggr(MV[1][:], BNS[1][:, 24:48])
                    sxs = cst.tile([128, 1], F32, tag="sxs")
                    sqs = cst.tile([128, 1], F32, tag="sqs")
                    nc.vector.reduce_sum(sxs[:], SX[:], axis=AX.X)
                    nc.vector.reduce_sum(sqs[:], SQ[:], axis=AX.X)
                    # mean_p = 0.5*mean_bn + sxs/4096
                    nc.vector.tensor_scalar_mul(ST[1][:, 0:1], MV[1][:, 0:1], 0.5)
                    nc.vector.tensor_scalar_mul(sxs[:], sxs[:], 1.0 / 4096.0)
                    nc.vector.tensor_add(ST[1][:, 0:1], ST[1][:, 0:1], sxs[:])
                    # E2_p = 0.5*(var_bn + mean_bn^2) + sqs/4096
                    nc.vector.tensor_mul(ST[1][:, 1:2], MV[1][:, 0:1], MV[1][:, 0:1])
                    nc.vector.tensor_add(ST[1][:, 1:2], ST[1][:, 1:2], MV[1][:, 1:2])
                    nc.vector.tensor_scalar_mul(ST[1][:, 1:2], ST[1][:, 1:2], 0.5)
                    nc.vector.tensor_scalar_mul(sqs[:], sqs[:], 1.0 / 4096.0)
                    nc.vector.tensor_add(ST[1][:, 1:2], ST[1][:, 1:2], sqs[:])
                nc.tensor.matmul(gs_ps[:, 2 * cc:2 * cc + 2], SEL,
                                 ST[cc][:], start=True, stop=True)
                nc.vector.tensor_copy(GS[:, 2 * cc:2 * cc + 2],
                                      gs_ps[:, 2 * cc:2 * cc + 2])

            # per-channel scale/shift: s = gamma/sqrt(var+eps), t = beta - mean*s
            gn_st = []
            for cc in range(2):
                pc_ps = po.tile([128, 2], F32, tag="po")
                nc.tensor.matmul(pc_ps[:], SELT[:], GS[:, 2 * cc:2 * cc + 2],
                                 start=True, stop=True)
                mean = cst.tile([128, 1], F32, tag=f"mean{cc}")
                var = cst.tile([128, 1], F32, tag=f"var{cc}")
                sd = cst.tile([128, 1], F32, tag=f"sd{cc}")
                s_t = cst.tile([128, 1], F32, tag=f"s{cc}")
                t_t = cst.tile([128, 1], F32, tag=f"t{cc}")
                nc.vector.tensor_scalar_mul(mean[:], pc_ps[:, 0:1], INV_GN)
                nc.vector.tensor_scalar_mul(var[:], pc_ps[:, 1:2], INV_GN)
                # var = E[x^2] - mean^2
                nc.vector.scalar_tensor_tensor(
                    out=sd[:], in0=mean[:], scalar=-1.0, in1=mean[:],
                    op0=ALU.mult, op1=ALU.mult)
                nc.vector.tensor_add(var[:], var[:], sd[:])
                nc.scalar.activation(sd[:], var[:], AF.Sqrt, bias=EPS[:])
                nc.vector.reciprocal(s_t[:], sd[:])
                nc.vector.tensor_mul(s_t[:], s_t[:], GAM[:, cc:cc + 1])
                nc.vector.scalar_tensor_tensor(
                    out=t_t[:], in0=mean[:], scalar=-1.0, in1=s_t[:],
                    op0=ALU.mult, op1=ALU.mult)
                nc.vector.tensor_add(t_t[:], t_t[:], BET[:, cc:cc + 1])
                gn_st.append((s_t, t_t))

            # h = x*s + t (bf16): all on DVE -- all-SBUF 2-byte TensorScalar
            # hits the 4x DVE mode (~327ns/slice)
            nc.scalar.activation(WARM[:], WARM[:], AF.Exp)  # preload exp table
            def emit_hb(i):
                sl = slice(i * 1024, (i + 1) * 1024)
                for cc in range(2):
                    s_t, t_t = gn_st[cc]
                    nc.vector.tensor_scalar(
                        out=Hb[cc][:, sl], in0=X[cc][:, sl], scalar1=s_t[:],
                        scalar2=t_t[:], op0=ALU.mult, op1=ALU.add)
            emit_hb(0)

            # ---- QKV ----
            # Q2/K2: fp8-e4m3, head-dim split across 2 DoubleRow k-subtiles:
            # partition r (0..31), free = h*8192 + t*4096 + token, where the
            # original feature index is h*64 + t*32 + r. QK then runs as one
            # dual-pumped fp8 matmul per (kt, head): 107ns instead of 427.
            Q2 = big.tile([32, 4 * N], E4, tag="q2")
            K2 = big.tile([32, 4 * N], E4, tag="k2")
            VT = big.tile([128, NKT * 144], E5, tag="vt")

            # staging holds a PAIR of 512-token chunks -> 4 shuffle DMAs per
            # pair instead of per chunk
            Q8 = [big.tile([128, 2 * QC], E4, tag=f"q8_{i}", name=f"Q8_{i}")
                  for i in range(2)]
            K8 = [big.tile([128, 2 * QC], E4, tag=f"k8_{i}", name=f"K8_{i}")
                  for i in range(2)]

            def _shuffle2(dst2, stage, chpair, q):
                # (128, 1024) staging of chunks {2c, 2c+1} -> (32, [h][t])
                for h in range(2):
                    for t in range(2):
                        off = (h * 2 + t) * N + chpair * 2 * QC
                        q.dma_start(dst2[:, off:off + 2 * QC],
                                    stage[h * 64 + t * 32:h * 64 + t * 32 + 32, :])

            qs_map = {"sync": nc.sync, "gpsimd": nc.gpsimd}

            def _mover(stage_sl, src_ps, bias_ap):
                if KNOBS["qk_mover"] == "dve":
                    nc.vector.tensor_scalar(out=stage_sl, in0=src_ps,
                                            scalar1=bias_ap, scalar2=None,
                                            op0=ALU.add)
                elif KNOBS["qk_mover"] == "pool":
                    nc.gpsimd.tensor_scalar(out=stage_sl, in0=src_ps,
                                            scalar1=bias_ap, scalar2=None,
                                            op0=ALU.add)
                else:
                    nc.scalar.activation(stage_sl, src_ps, AF.Identity,
                                         bias=bias_ap)

            def emit_q_pair(chpair):
                stage = Q8[chpair % 2]
                for sub in range(2):
                    ch = 2 * chpair + sub
                    tok = slice(ch * QC, (ch + 1) * QC)
                    q_ps = ps.tile([128, QC], F32, tag="s", name=f"q_ps{ch}")
                    nc.tensor.matmul(q_ps[:], W0[:, 0:128], Hb[0][:, tok],
                                     start=True, stop=False)
                    nc.tensor.matmul(q_ps[:], W1[:, 0:128], Hb[1][:, tok],
                                     start=False, stop=True)
                    _mover(stage[:, sub * QC:(sub + 1) * QC], q_ps[:],
                           BQK[:, 0:1])
                _shuffle2(Q2, stage, chpair, qs_map[KNOBS["qshq"]])

            def emit_k_pair(chpair):
                stage = K8[chpair % 2]
                for sub in range(2):
                    ch = 2 * chpair + sub
                    tok = slice(ch * QC, (ch + 1) * QC)
                    k_ps = ps.tile([128, QC], F32, tag="s", name=f"k_ps{ch}")
                    nc.tensor.matmul(k_ps[:], W0[:, 128:256], Hb[0][:, tok],
                                     start=True, stop=False)
                    nc.tensor.matmul(k_ps[:], W1[:, 128:256], Hb[1][:, tok],
                                     start=False, stop=True)
                    _mover(stage[:, sub * QC:(sub + 1) * QC], k_ps[:],
                           BQK[:, 1:2])
                _shuffle2(K2, stage, chpair, qs_map[KNOBS["kshq"]])

            def emit_vt_tile(kt):
                tok = slice(kt * KT, (kt + 1) * KT)
                vt_ps = ps.tile([128, 144], F32, tag="s", name=f"vt_ps{kt}")
                nc.tensor.matmul(vt_ps[:], Hb[0][:, tok], W0[:, 256:400],
                                 start=True, stop=False)
                nc.tensor.matmul(vt_ps[:], Hb[1][:, tok], W1[:, 256:400],
                                 start=False, stop=False)
                # V bias (+ the denominator 1s column) via rank-1 accumulate
                nc.tensor.matmul(vt_ps[:], ONEB[0:1, :], BV[:],
                                 start=False, stop=True)
                nc.gpsimd.tensor_copy(VT[:, kt * 144:(kt + 1) * 144], vt_ps[:])

            # chunk-pair-0 QKV only needs Hb cols 0:1024 -- start right after
            # the first Hb slice, then fill in the rest of Hb
            emit_q_pair(0)
            emit_k_pair(0)
            for i in range(1, 4):
                emit_hb(i)

            # ---- attention + per-head unnormalized projection ----
            ENG = {"act": None, "dve": nc.vector, "pool": nc.gpsimd}

            def _copy(eng, dst, src):
                if eng == "act":
                    nc.scalar.activation(dst, src, AF.Copy)
                elif eng == "dve":
                    nc.vector.tensor_copy(dst, src)
                else:
                    nc.gpsimd.tensor_copy(dst, src)

            pending = []     # finish ops of the previous qc, one per sub slot
            pv_queue = []
            for qc in range(NQC):
                O_A = po.tile([72, QC], F32, tag="po", name=f"O_A{qc}")
                O_B = po.tile([72, QC], F32, tag="po", name=f"O_B{qc}")
                mypat = pat0 if qc == 0 else pat
                for pr in range(NPR):
                    P8 = pp.tile([128, 2048], E5, tag="p", name=f"p{qc}_{pr}")
                    for sub in range(2):
                        kt = 2 * pr + sub
                        if qc == 0:
                            if kt % 8 == 4 and kt // 8 < 3:
                                emit_k_pair(kt // 8 + 1)
                            if kt < KNOBS["vt_split"]:
                                emit_vt_tile(kt)
                        if qc == 1 and kt < NKT - KNOBS["vt_split"]:
                            emit_vt_tile(kt + KNOBS["vt_split"])
                        if pr <= 1 and pv_queue:
                            pv_queue.pop(0)()
                        si = 2 * pr + sub
                        for pos, op in list(pending):
                            if si >= pos:
                                op()
                                pending.remove((pos, op))
                        if si == KNOBS["q_pos"] and qc % 2 == 0 and qc // 2 + 1 < 4:
                            emit_q_pair(qc // 2 + 1)
                        if pr >= KNOBS["pop_lo"] and len(pv_queue) >= KNOBS["pop_len"]:
                            pv_queue.pop(0)()
                        s_h = [ps.tile([128, 512], F32, tag="s",
                                       name=f"s{qc}_{kt}_{h}")
                               for h in range(2)]
                        if qc == 0 and kt < 8:
                            # warmup: the un-shuffled e4m3 staging tiles still
                            # hold these key chunks -- plain fp8 matmuls, no
                            # wait on the shuffle DMAs
                            ks8 = K8[0]
                            kk = slice(kt * KT, (kt + 1) * KT)
                            nc.tensor.matmul(s_h[0][:], ks8[0:64, kk],
                                             Q8[0][0:64, 0:QC], start=True,
                                             stop=True)
                            nc.tensor.matmul(s_h[1][:], ks8[64:128, kk],
                                             Q8[0][64:128, 0:QC], start=True,
                                             stop=True)
                        else:
                            k2v = K2[:].rearrange("p (h t c) -> h p t c", h=2, t=2)[
                                :, :, :, kt * KT:(kt + 1) * KT]
                            q2v = Q2[:].rearrange("p (h t c) -> h p t c", h=2, t=2)[
                                :, :, :, qc * QC:(qc + 1) * QC]
                            nc.tensor.matmul(s_h[0][:], k2v[0], q2v[0],
                                             start=True, stop=True, perf_mode=DR)
                            nc.tensor.matmul(s_h[1][:], k2v[1], q2v[1],
                                             start=True, stop=True, perf_mode=DR)
                        for h in range(2):
                            dst = P8[:, sub * 1024 + h * 512:
                                     sub * 1024 + (h + 1) * 512]
                            eng = mypat[2 * kt + h]
                            if eng == "dve":
                                nc.vector.tensor_scalar(
                                    out=dst.bitcast(U8), in0=s_h[h][:],
                                    scalar1=A_SCH, scalar2=B_SCH,
                                    op0=ALU.mult, op1=ALU.add)
                            elif eng == "pool":
                                nc.gpsimd.tensor_scalar(
                                    out=dst.bitcast(U8), in0=s_h[h][:],
                                    scalar1=A_SCH, scalar2=B_SCH,
                                    op0=ALU.mult, op1=ALU.add)
                            else:
                                nc.scalar.activation(dst, s_h[h][:], AF.Exp,
                                                     scale=SCALE)

                    def _pv(pr=pr, P8=P8, O_A=O_A, O_B=O_B):
                        vt_ap = VT[:].rearrange("p (t x) -> p t x", t=NKT)[
                            :, 2 * pr:2 * pr + 2, :]
                        p_ap = P8[:].rearrange("p (t x) -> p t x", t=2)
                        nc.tensor.matmul(O_A[:], vt_ap[:, :, 0:72],
                                         p_ap[:, :, 0:512],
                                         start=(pr == 0), stop=(pr == NPR - 1),
                                         perf_mode=DR)
                        nc.tensor.matmul(O_B[:], vt_ap[:, :, 72:144],
                                         p_ap[:, :, 512:1024],
                                         start=(pr == 0), stop=(pr == NPR - 1),
                                         perf_mode=DR)
                    pv_queue.append(_pv)

                OsbA = sm.tile([64, QC], BF16, tag="osba", name=f"OsbA{qc}")
                OsbB = sm.tile([64, QC], BF16, tag="osbb", name=f"OsbB{qc}")

                def fa0(qc=qc, O_A=O_A, OsbA=OsbA):
                    _copy(KNOBS["o_eng"], OsbA[:], O_A[0:64, :])

                def fa1(qc=qc, O_B=O_B, OsbB=OsbB):
                    _copy(KNOBS["o_eng"], OsbB[:], O_B[0:64, :])

                def fdn(qc=qc, O_A=O_A, O_B=O_B):
                    dsl = slice(qc * QC, (qc + 1) * QC)
                    _copy(KNOBS["dn_eng"], DN[0:1, dsl], O_A[64:65, :])
                    _copy(KNOBS["dn_eng"], DN[1:2, dsl], O_B[64:65, :])

                def fy(half, qc=qc, OsbA=OsbA, OsbB=OsbB):
                    cols = slice(half * 128, (half + 1) * 128)
                    y_psA = ps.tile([128, 512], F32, tag="s",
                                    name=f"y_ps{qc}_{half}a")
                    y_psB = ps.tile([128, 512], F32, tag="s",
                                    name=f"y_ps{qc}_{half}b")
                    nc.tensor.matmul(y_psA[:], WPA[:, cols],
                                     OsbA[:], start=True, stop=True)
                    nc.tensor.matmul(y_psB[:], WPB[:, cols],
                                     OsbB[:], start=True, stop=True)
                    y_sb = yy.tile([128, 1024], F32, tag="y",
                                   name=f"y_sb{qc}_{half}")
                    ye = KNOBS["y_eng"]
                    if ye == "split":
                        yeA, yeB = "act", "dve"
                    else:
                        yeA = yeB = ye
                    _copy(yeA, y_sb[:, 0:512], y_psA[:])
                    _copy(yeB, y_sb[:, 512:1024], y_psB[:])
                    nc.sync.dma_start(yp.ap()[half * 8 + qc], y_sb[:])

                fa, fb = KNOBS["fa_pos"], KNOBS["fb_pos"]
                mine = [(fa, fa0), (fa + 1, fa1), (fa + 2, fdn),
                        (fb, lambda qc=qc, OsbA=OsbA, OsbB=OsbB: fy(0, qc, OsbA, OsbB)),
                        (fb + 1, lambda qc=qc, OsbA=OsbA, OsbB=OsbB: fy(1, qc, OsbA, OsbB))]
                if qc < NQC - 1:
                    pending = mine
                else:
                    while pv_queue:
                        pv_queue.pop(0)()
                    for _, op in mine:
                        op()
            nc.sync.dma_start(dnp.ap(), DN[:])

    nc.compile()
    return nc


def _get_nc():
    if "nc" not in _CACHE:
        _CACHE["nc"] = _build()
    return _CACHE["nc"]


def build_in_maps(x, gn_gamma, gn_beta, w_qkv, b_qkv, w_proj):
    import ml_dtypes
    sel_np = np.zeros((128, 4), np.float32)
    for c in range(128):
        sel_np[c, c // 32] = 1.0
    selt_np = sel_np.T.copy()
    gmt_np = np.stack([gn_gamma[0:128], gn_gamma[128:256]], axis=1)
    btt_np = np.stack([gn_beta[0:128], gn_beta[128:256]], axis=1)

    in_maps = []
    for core in range(8):
        b, j = core // 2, core % 2
        r0 = 128 * j
        wsl_np = np.zeros((2, 128, 400), np.float32)
        for cc in range(2):
            cols = slice(cc * 128, (cc + 1) * 128)
            wsl_np[cc, :, 0:128] = w_qkv[r0:r0 + 128, cols].T
            wsl_np[cc, :, 128:256] = w_qkv[256 + r0:256 + r0 + 128, cols].T
            wsl_np[cc, :, 256:320] = w_qkv[512 + r0:512 + r0 + 64, cols].T
            wsl_np[cc, :, 328:392] = w_qkv[512 + r0 + 64:512 + r0 + 128, cols].T
        bqk_np = np.stack([b_qkv[r0:r0 + 128], b_qkv[256 + r0:256 + r0 + 128]],
                          axis=1)
        bv_np = np.zeros((1, 144), np.float32)
        bv_np[0, 0:64] = b_qkv[512 + r0:512 + r0 + 64]
        bv_np[0, 64] = 1.0
        bv_np[0, 72:136] = b_qkv[512 + r0 + 64:512 + r0 + 128]
        bv_np[0, 136] = 1.0
        csts_np = np.concatenate([bqk_np, gmt_np, btt_np, sel_np], axis=1)
        xq = np.ascontiguousarray(
            x[b].reshape(2, 128, 4, 1024).transpose(0, 2, 1, 3)
            .reshape(8, 128, 1024).astype(ml_dtypes.bfloat16))
        in_maps.append({
            "xb": xq,
            "wslb": np.ascontiguousarray(wsl_np.astype(ml_dtypes.bfloat16)),
            "csts": np.ascontiguousarray(csts_np),
            "bv16": np.ascontiguousarray(bv_np.astype(ml_dtypes.bfloat16)),
            "selt": selt_np,
            "wpt": np.ascontiguousarray(
                w_proj[:, r0:r0 + 128].T.astype(ml_dtypes.bfloat16)),
        })

    return in_maps


def kernel(x, gn_gamma, gn_beta, w_qkv, b_qkv, w_proj, b_proj, **_unused):
    x = np.ascontiguousarray(np.asarray(x, dtype=np.float32))
    gn_gamma = np.asarray(gn_gamma, dtype=np.float32)
    gn_beta = np.asarray(gn_beta, dtype=np.float32)
    w_qkv = np.asarray(w_qkv, dtype=np.float32)
    b_qkv = np.asarray(b_qkv, dtype=np.float32)
    w_proj = np.asarray(w_proj, dtype=np.float32)
    b_proj = np.asarray(b_proj, dtype=np.float32)

    nc = _get_nc()
    in_maps = build_in_maps(x, gn_gamma, gn_beta, w_qkv, b_qkv, w_proj)
    res = bass_utils.run_bass_kernel_spmd(nc, in_maps, core_ids=list(range(8)))
    _CACHE["last_result"] = res

    out = np.empty((B, C, N), np.float32)
    for b in range(B):
        acc = np.zeros((C, N), np.float32)
        for j in range(2):
            r = res.results[2 * b + j]
            ypj = r["yp"].reshape(2, 8, 128, 2, 512)   # half, qc, ch, head, tok
            rden = 1.0 / r["dnp"]                       # (2, 4096) head, tok
            # per head: (half, qc, ch, tok) -> (C, N), scaled by 1/denom
            for head in range(2):
                yh = ypj[:, :, :, head, :]              # (2, 8, 128, 512)
                yh = yh.transpose(0, 2, 1, 3).reshape(C, N)
                acc += yh * rden[head][None, :]
        out[b] = acc + x[b].reshape(C, N) + b_proj[:, None]
    return out.reshape(B, C, H, W)
